# revision 1
# baseline (speedup 1.0000x reference)
"""Trainium2 Bass kernel for the decoder KV-cache scatter update.

Reference semantics (per layer l, batch b, head h):
    k_upd = k_cache;  k_upd[l, b, h, position_ids[b], :] = new_k[l, b, h, 0, :]
    v_upd = v_cache;  v_upd[l, b, h, position_ids[b], :] = new_v[l, b, h, 0, :]
    mask  = attention_mask[:, None, None, :].astype(bool)

Sharding: the layer axis L == 8 == n_cores, so core l owns layer l.
Per core the work is two 32 MiB DRAM->DRAM bulk copies (HWDGE, all 16
SDMA engines) followed by an indirect-DMA scatter of 16 rows (512 B
each) whose row indices are computed on the host from position_ids
(slot-mapping style — the compiled NEFF itself is input-independent).
"""

import numpy as np

import concourse.bass as bass
import concourse.mybir as mybir
from concourse.bass_utils import run_bass_kernel_spmd

# Problem shapes (hardcoded per harness contract).
L, B, H, S, D = 8, 2, 8, 4096, 128
N_CORES = 8
R = B * H * S          # rows of one layer's cache, flattened: 65536
NK = B * H             # scatter rows per layer: 16
M = B * S              # mask elements: 8192

# Results of the most recent kernel() call (for test harness inspection).
LAST_RESULTS = None

_NC_CACHE = None


def _build_nc():
    """One-layer SPMD program: bulk-copy k/v, scatter NK new rows, cast mask."""
    nc = bass.Bass()
    f32 = mybir.dt.float32
    i32 = mybir.dt.int32
    u8 = mybir.dt.uint8

    k_in = nc.declare_dram_parameter("k_in", [R, D], f32, isOutput=False)
    v_in = nc.declare_dram_parameter("v_in", [R, D], f32, isOutput=False)
    nk_in = nc.declare_dram_parameter("nk_in", [NK, D], f32, isOutput=False)
    nv_in = nc.declare_dram_parameter("nv_in", [NK, D], f32, isOutput=False)
    am_in = nc.declare_dram_parameter("am_in", [M], i32, isOutput=False)
    idx_in = nc.declare_dram_parameter("idx_in", [NK, 1], i32, isOutput=False)

    k_out = nc.declare_dram_parameter("k_out", [R, D], f32, isOutput=True)
    v_out = nc.declare_dram_parameter("v_out", [R, D], f32, isOutput=True)
    m_out = nc.declare_dram_parameter("m_out", [M], u8, isOutput=True)

    with (
        nc.sbuf_tensor([NK, D], f32) as nk_sb,
        nc.sbuf_tensor([NK, D], f32) as nv_sb,
        nc.sbuf_tensor([NK, 1], i32) as idx_sb,
        nc.semaphore("bulk_sem") as bulk_sem,
        nc.semaphore("ld_sem") as ld_sem,
        nc.semaphore("sc_sem") as sc_sem,
        nc.Block() as block,
    ):
        @block.sync
        def _(sync):
            sync.dma_start(out=k_out[:], in_=k_in[:]).then_inc(bulk_sem, 16)
            sync.dma_start(out=v_out[:], in_=v_in[:]).then_inc(bulk_sem, 16)

        @block.gpsimd
        def _(gpsimd):
            gpsimd.dma_start(out=nk_sb[:], in_=nk_in[:]).then_inc(ld_sem, 16)
            gpsimd.dma_start(out=nv_sb[:], in_=nv_in[:]).then_inc(ld_sem, 16)
            gpsimd.dma_start(out=idx_sb[:], in_=idx_in[:]).then_inc(ld_sem, 16)
            # mask: int32 -> uint8 cast during DMA (SWDGE-only feature)
            gpsimd.dma_start(out=m_out[:], in_=am_in[:]).then_inc(ld_sem, 16)
            gpsimd.wait_ge(ld_sem, 64)
            gpsimd.wait_ge(bulk_sem, 32)
            gpsimd.indirect_dma_start(
                out=k_out[:],
                out_offset=bass.IndirectOffsetOnAxis(ap=idx_sb[:, :1], axis=0),
                in_=nk_sb[:],
                in_offset=None,
            ).then_inc(sc_sem, 16)
            gpsimd.indirect_dma_start(
                out=v_out[:],
                out_offset=bass.IndirectOffsetOnAxis(ap=idx_sb[:, :1], axis=0),
                in_=nv_sb[:],
                in_offset=None,
            ).then_inc(sc_sem, 16)
            gpsimd.wait_ge(sc_sem, 32)

    return nc


def kernel(k_cache, v_cache, new_k, new_v, attention_mask, position_ids):
    global LAST_RESULTS, _NC_CACHE

    k_cache = np.ascontiguousarray(np.asarray(k_cache, dtype=np.float32))
    v_cache = np.ascontiguousarray(np.asarray(v_cache, dtype=np.float32))
    new_k = np.ascontiguousarray(np.asarray(new_k, dtype=np.float32))
    new_v = np.ascontiguousarray(np.asarray(new_v, dtype=np.float32))
    attention_mask = np.ascontiguousarray(np.asarray(attention_mask, dtype=np.int32))
    position_ids = np.asarray(position_ids, dtype=np.int32)

    # Host-computed scatter row indices into the [R, D] per-layer view:
    # row(b, h) = (b*H + h)*S + position_ids[b]  (slot mapping).
    pos = position_ids.reshape(B).astype(np.int64)
    bh = np.arange(B * H, dtype=np.int64)
    idx = (bh * S + np.repeat(pos, H)).astype(np.int32).reshape(NK, 1)

    am_flat = attention_mask.reshape(M)

    in_maps = []
    for l in range(N_CORES):
        in_maps.append({
            "k_in": k_cache[l].reshape(R, D),
            "v_in": v_cache[l].reshape(R, D),
            "nk_in": new_k[l].reshape(NK, D),
            "nv_in": new_v[l].reshape(NK, D),
            "am_in": am_flat,
            "idx_in": idx,
        })

    if _NC_CACHE is None:
        _NC_CACHE = _build_nc()
    res = run_bass_kernel_spmd(_NC_CACHE, in_maps, list(range(N_CORES)))
    LAST_RESULTS = res

    k_upd = np.stack([res.results[c]["k_out"] for c in range(N_CORES)]) \
        .reshape(L, B, H, S, D)
    v_upd = np.stack([res.results[c]["v_out"] for c in range(N_CORES)]) \
        .reshape(L, B, H, S, D)
    mask = res.results[0]["m_out"].reshape(B, 1, 1, S) != 0

    return mask, k_upd, v_upd


# revision 5
# speedup vs baseline: 11.5976x; 11.5976x over previous
"""Trainium2 Bass kernel for the decoder KV-cache scatter update.

Reference semantics (per layer l, batch b, head h):
    k_upd = k_cache;  k_upd[l, b, h, position_ids[b], :] = new_k[l, b, h, 0, :]
    v_upd = v_cache;  v_upd[l, b, h, position_ids[b], :] = new_v[l, b, h, 0, :]
    mask  = attention_mask[:, None, None, :].astype(bool)

Sharding: the layer axis L == 8 == n_cores, so core l owns layer l.
Per core the work is two 32 MiB DRAM->DRAM bulk copies (HWDGE, all 16
SDMA engines) followed by an indirect-DMA scatter of 16 rows (512 B
each) whose row indices are computed on the host from position_ids
(slot-mapping style — the compiled NEFF itself is input-independent).
"""

import numpy as np

import concourse.bass as bass
import concourse.mybir as mybir
from concourse.bass_utils import run_bass_kernel_spmd

# Problem shapes (hardcoded per harness contract).
L, B, H, S, D = 8, 2, 8, 4096, 128
N_CORES = 8
R = B * H * S          # rows of one layer's cache, flattened: 65536
NK = B * H             # scatter rows per layer: 16
M = B * S              # mask elements: 8192

# Results of the most recent kernel() call (for test harness inspection).
LAST_RESULTS = None

_NC_CACHE = None


def _build_nc(repeat=1):
    """One-layer SPMD program: bulk-copy k/v, scatter NK new rows, cast mask.

    repeat > 1 re-issues the bulk copies (idempotent) so a test harness can
    measure per-iteration device time as a slope, cancelling dispatch
    overhead. kernel() always uses repeat=1.
    """
    nc = bass.Bass()
    f32 = mybir.dt.float32
    i32 = mybir.dt.int32
    u8 = mybir.dt.uint8

    k_in = nc.declare_dram_parameter("k_in", [R, D], f32, isOutput=False)
    v_in = nc.declare_dram_parameter("v_in", [R, D], f32, isOutput=False)
    nk_in = nc.declare_dram_parameter("nk_in", [NK, D], f32, isOutput=False)
    nv_in = nc.declare_dram_parameter("nv_in", [NK, D], f32, isOutput=False)
    am_in = nc.declare_dram_parameter("am_in", [M], i32, isOutput=False)
    idx_in = nc.declare_dram_parameter("idx_in", [NK, 1], i32, isOutput=False)

    k_out = nc.declare_dram_parameter("k_out", [R, D], f32, isOutput=True)
    v_out = nc.declare_dram_parameter("v_out", [R, D], f32, isOutput=True)
    m_out = nc.declare_dram_parameter("m_out", [M], u8, isOutput=True)

    with (
        nc.sbuf_tensor([NK, D], f32) as nk_sb,
        nc.sbuf_tensor([NK, D], f32) as nv_sb,
        nc.sbuf_tensor([NK, 1], i32) as idx_sb,
        nc.semaphore("bulk_sem") as bulk_sem,
        nc.semaphore("ld_sem") as ld_sem,
        nc.semaphore("sc_sem") as sc_sem,
        nc.Block() as block,
    ):
        # One bulk DRAM->DRAM copy per HWDGE ring: k on qSPDynamicHW (sync),
        # v on qActDynamicHW (scalar). A single ring tops out ~330 GB/s of
        # HBM traffic per core; two rings together reach the HBM roofline.
        @block.sync
        def _(sync):
            for _r in range(repeat):
                sync.dma_start(out=k_out[:], in_=k_in[:]).then_inc(bulk_sem, 16)

        @block.scalar
        def _(scalar):
            for _r in range(repeat):
                scalar.dma_start(out=v_out[:], in_=v_in[:]).then_inc(bulk_sem, 16)

        @block.gpsimd
        def _(gpsimd):
            gpsimd.dma_start(out=nk_sb[:], in_=nk_in[:]).then_inc(ld_sem, 16)
            gpsimd.dma_start(out=nv_sb[:], in_=nv_in[:]).then_inc(ld_sem, 16)
            gpsimd.dma_start(out=idx_sb[:], in_=idx_in[:]).then_inc(ld_sem, 16)
            # mask: int32 -> uint8 cast during DMA (SWDGE-only feature)
            gpsimd.dma_start(out=m_out[:], in_=am_in[:]).then_inc(ld_sem, 16)
            gpsimd.wait_ge(ld_sem, 64)
            gpsimd.wait_ge(bulk_sem, 32 * repeat)
            gpsimd.indirect_dma_start(
                out=k_out[:],
                out_offset=bass.IndirectOffsetOnAxis(ap=idx_sb[:, :1], axis=0),
                in_=nk_sb[:],
                in_offset=None,
            ).then_inc(sc_sem, 16)
            gpsimd.indirect_dma_start(
                out=v_out[:],
                out_offset=bass.IndirectOffsetOnAxis(ap=idx_sb[:, :1], axis=0),
                in_=nv_sb[:],
                in_offset=None,
            ).then_inc(sc_sem, 16)
            gpsimd.wait_ge(sc_sem, 32)

    return nc


def kernel(k_cache, v_cache, new_k, new_v, attention_mask, position_ids):
    global LAST_RESULTS, _NC_CACHE

    k_cache = np.ascontiguousarray(np.asarray(k_cache, dtype=np.float32))
    v_cache = np.ascontiguousarray(np.asarray(v_cache, dtype=np.float32))
    new_k = np.ascontiguousarray(np.asarray(new_k, dtype=np.float32))
    new_v = np.ascontiguousarray(np.asarray(new_v, dtype=np.float32))
    attention_mask = np.ascontiguousarray(np.asarray(attention_mask, dtype=np.int32))
    position_ids = np.asarray(position_ids, dtype=np.int32)

    # Host-computed scatter row indices into the [R, D] per-layer view:
    # row(b, h) = (b*H + h)*S + position_ids[b]  (slot mapping).
    pos = position_ids.reshape(B).astype(np.int64)
    bh = np.arange(B * H, dtype=np.int64)
    idx = (bh * S + np.repeat(pos, H)).astype(np.int32).reshape(NK, 1)

    am_flat = attention_mask.reshape(M)

    in_maps = []
    for l in range(N_CORES):
        in_maps.append({
            "k_in": k_cache[l].reshape(R, D),
            "v_in": v_cache[l].reshape(R, D),
            "nk_in": new_k[l].reshape(NK, D),
            "nv_in": new_v[l].reshape(NK, D),
            "am_in": am_flat,
            "idx_in": idx,
        })

    if _NC_CACHE is None:
        _NC_CACHE = _build_nc()
    res = run_bass_kernel_spmd(_NC_CACHE, in_maps, list(range(N_CORES)))
    LAST_RESULTS = res

    k_upd = np.stack([res.results[c]["k_out"] for c in range(N_CORES)]) \
        .reshape(L, B, H, S, D)
    v_upd = np.stack([res.results[c]["v_out"] for c in range(N_CORES)]) \
        .reshape(L, B, H, S, D)
    mask = res.results[0]["m_out"].reshape(B, 1, 1, S) != 0

    return mask, k_upd, v_upd
